# revision 48
# baseline (speedup 1.0000x reference)
"""Distributed Bass kernel for nn_Attention_16509854286348.

Strategy (8 NeuronCores, tensor-parallel over heads):
  - Each core owns 2 of the 16 heads: it computes q/k/v projections for
    its 256 output features from the (replicated) input x, applies
    RMSNorm + RoPE (norm weights and the 1/sqrt(dh) attention scale are
    folded into the rope factor tables on the host), runs attention for
    its (batch, head) pairs, and produces O^T [256, tok] slices.
  - The O^T slices are AllGathered in per-(batch, q-tile) chunks.
    Phase order proj0 -> attn0 -> proj1 -> attn1 (per-batch q/k/v state
    is double-buffered) starts the AllGather stream ~120us earlier than
    attention-at-the-end would; batch 0's wo chunks run inside proj1,
    batch 1's lag their chunk by 2 so their AllGathers have landed; the
    final chunks are small to shrink the exposed tail.
  - After each AllGather lands, the core computes a disjoint 256-row
    slice of the output projection, transposed ([f, tok]); the host
    transposes and concatenates the 8 slices.

Numerics: bf16 matmul operands with fp32 PSUM accumulation; the q/k
path keeps fp32 math through RMSNorm and RoPE and rounds to bf16 once
(at the rope output); softmax statistics fp32.  Scores are O(1) by
construction (RMS-normed q/k), so softmax skips the max subtraction.

Engine layout:
  - PE: projections as 384/384-column matmul pairs (the 128-column
    LDWEIGHTS for the next stationary operand hides under both), scores
    / PV / denominator matmuls at N=512, PE-transposes for the q/k
    blocks, output projection at N=512.
  - ACT: PSUM evictions in the projection phase; Exp in 3-block groups
    ([128,1536] reads across PSUM banks) to amortize the ~293ns fixed
    cost per activate.
  - DVE: rms stats (table-free Newton rsqrt), norm scale, rope,
    po/pden evictions, softmax normalize (reciprocal_approx_fast).
  - PSUM: 2 pools exactly filling 8 banks: "s3" [128,1536]x2 (proj
    accum + scores + exp source), "acc" [128,512]x2 (transpose quads
    in proj, po/pden in attention, wo accum).
"""

import os
import sys
import types

import numpy as np
import ml_dtypes

import concourse.bass as bass
import concourse.mybir as mybir
import concourse.tile as tile
from concourse.masks import make_identity

# ---------------------------------------------------------------------------
# Environment workarounds
# ---------------------------------------------------------------------------


def _patch_tile_drain():
    """walrus in this image rejects >1 sem wait on the TileContext exit
    drain ("Too many sync wait commands"); split the waits into
    individual single-wait nops on the sync engine."""
    import bass_rust
    from concourse import tile as _tile
    from concourse.vector_clock import ScopedClock

    if getattr(_tile.TileContext, "_drain_patched", False):
        return

    def _drain_and_barrier(self, tick_clock, wait_clock):
        nc = self.nc
        drain_inst = nc.sync.drain()
        wait_clock.add_sem_waits(
            drain_inst.ins, ScopedClock({None: tick_clock.global_clock})
        )
        si = drain_inst.ins.sync_info
        if si is not None and len(si.on_wait) > 1:
            waits = list(si.on_wait)
            updates = list(si.on_update)
            drain_inst.ins.sync_info = bass_rust.SyncInfo(
                on_wait=[], on_update=updates
            )
            for w in waits:
                n = nc.sync.nop(nofuse=True)
                n.ins.sync_info = bass_rust.SyncInfo(on_wait=[w], on_update=[])
        nc.all_engine_barrier()
        assert self.sems is not None
        popped = nc._tile_sem_poison_stack.pop()
        assert popped is self._sem_poison
        nc.clear_and_free_semaphores(list(self.sems.allocated().values()))
        nc.all_engine_barrier()

    _tile.TileContext._drain_and_barrier = _drain_and_barrier
    _tile.TileContext._drain_patched = True


def _legalize_waits(nc, max_waits=1):
    """This image's walrus rejects instructions with more than one sync
    wait ("Too many sync wait commands").  Hoist excess waits onto
    dedicated single-wait nops inserted just before the instruction on
    the same engine — semantically identical, since the engine stalls on
    the nops first."""
    import bass_rust

    counter = [0]
    for f in nc.m.functions:
        for bb in f.blocks:
            changed = False
            newlist = []
            for ins in bb.instructions:
                si = ins.sync_info
                if si is not None and len(si.on_wait) > max_waits:
                    waits = list(si.on_wait)
                    updates = list(si.on_update)
                    for w in waits[:-max_waits]:
                        counter[0] += 1
                        nop = mybir.InstNoOp(
                            name=f"LGW-{counter[0]}", ins=[], outs=[]
                        )
                        nop.engine = ins.engine
                        nop.sync_info = bass_rust.SyncInfo(
                            on_wait=[w], on_update=[]
                        )
                        newlist.append(nop)
                    ins.sync_info = bass_rust.SyncInfo(
                        on_wait=waits[-max_waits:], on_update=updates
                    )
                    changed = True
                newlist.append(ins)
            if changed:
                bb.instructions = newlist


def _patch_ldw_opt():
    """Turn on walrus's LDWEIGHTS optimization (hardcoded off in this
    image's bass_utils): rewrites the flag in the walrus_driver argv."""
    if os.environ.get("ATTN_LDW_OPT", "0") != "1":
        return
    import concourse.bass_utils as bu

    if getattr(bu, "_ldw_opt_patched", False):
        return
    orig = bu.run_command

    def run_command_ldw(argv, **kwargs):
        if isinstance(argv, list):
            argv = [
                "--enable-ldw-opt=true" if a == "--enable-ldw-opt=false" else a
                for a in argv
            ]
        return orig(argv, **kwargs)

    bu.run_command = run_command_ldw
    bu._ldw_opt_patched = True


def _strip_redundant_incs(nc, engine_insts=("InstMatmult",)):
    """Remove per-instruction semaphore increments that nothing waits on.

    The Tile framework increments a per-engine counting semaphore on
    every instruction; on the PE a serialized EVT_SEM write costs ~26ns
    per matmul (~6% of a N=512 matmul).  Engines complete instructions
    in program order, so a wait for "first k instructions done" is
    equivalent to "instruction #k done".  For each semaphore whose
    updaters are all same-engine instructions of the allowed types, keep
    increments only at the waited-on positions and renumber the wait
    thresholds to their rank.
    """
    import bass_rust

    all_ins = []
    for f in nc.m.functions:
        for bb in f.blocks:
            all_ins.extend(bb.instructions)

    updaters = {}   # sem id -> [(ins, engine, typename)]
    waits = {}      # sem id -> [(ins, SyncWait)]
    bad = set()     # sems we must not touch
    for ins in all_ins:
        si = ins.sync_info
        if si is None:
            continue
        for u in si.on_update:
            if (
                u.sync_type == "semaphore"
                and u.update_mode == "sem-inc"
                and u.update_value == 1
            ):
                updaters.setdefault(u.id, []).append(
                    (ins, ins.engine, type(ins).__name__)
                )
            else:
                bad.add(u.id)
        for w in si.on_wait:
            if w.sync_type == "semaphore":
                if w.wait_mode != "sem-ge-imm":
                    bad.add(w.id)
                else:
                    waits.setdefault(w.id, []).append((ins, w))

    for sem_id, ups in updaters.items():
        if sem_id in bad:
            continue
        engines = {e for _, e, _ in ups}
        types = {t for _, _, t in ups}
        if len(engines) != 1 or not types.issubset(set(engine_insts)):
            continue
        ks = sorted({w.wait_value for _, w in waits.get(sem_id, [])})
        if ks and (ks[0] < 1 or ks[-1] > len(ups)):
            continue  # threshold out of range: scheme not understood; skip
        rank = {k: i + 1 for i, k in enumerate(ks)}
        keep = set(ks)
        for pos, (ins, _, _) in enumerate(ups, start=1):
            if pos in keep:
                continue
            si = ins.sync_info
            ups_new = [
                u
                for u in si.on_update
                if not (u.sync_type == "semaphore" and u.id == sem_id)
            ]
            ins.sync_info = bass_rust.SyncInfo(
                on_wait=list(si.on_wait), on_update=ups_new
            )
        seen = set()
        for ins, w in waits.get(sem_id, []):
            if id(ins) in seen:
                continue
            seen.add(id(ins))
            si = ins.sync_info
            new_waits = []
            for ww in si.on_wait:
                if ww.sync_type == "semaphore" and ww.id == sem_id:
                    new_waits.append(
                        bass_rust.SyncWait(
                            sync_type=ww.sync_type,
                            id=ww.id,
                            ant_name=ww.ant_name,
                            wait_mode=ww.wait_mode,
                            wait_value=rank[ww.wait_value],
                            wait_reg=ww.wait_reg,
                        )
                    )
                else:
                    new_waits.append(ww)
            ins.sync_info = bass_rust.SyncInfo(
                on_wait=new_waits, on_update=list(si.on_update)
            )


def _register_ntff_hook():
    """The image's antenv package lacks axon_hooks; supply it so
    run_bass_kernel_spmd(trace=True) can profile under axon."""
    if "antenv.axon_hooks" in sys.modules:
        return
    import antenv

    mod = types.ModuleType("antenv.axon_hooks")
    mod._hook = None

    def set_axon_ntff_profile_hook(h):
        mod._hook = h

    def get_axon_ntff_profile_hook():
        return mod._hook

    mod.set_axon_ntff_profile_hook = set_axon_ntff_profile_hook
    mod.get_axon_ntff_profile_hook = get_axon_ntff_profile_hook
    sys.modules["antenv.axon_hooks"] = mod
    antenv.axon_hooks = mod
    try:
        from trn_agent_boot.trn_boot import _ntff_profile_via_ctypes

        mod.set_axon_ntff_profile_hook(
            _ntff_profile_via_ctypes("/opt/axon/libaxon_pjrt.so")
        )
    except Exception:
        pass


# ---------------------------------------------------------------------------
# Problem constants (hardcoded per spec)
# ---------------------------------------------------------------------------

B, S, DM = 2, 2048, 2048
H, DH = 16, 128
EPS = 1e-6
NCORES = 8
HL = H // NCORES            # heads per core = 2
FC = HL * DH                # feature slice per core = 256
TOK = B * S                 # 4096
SB = S // 128               # 16 seq blocks per batch
KB = S // 128               # 16 key blocks per batch

F32 = mybir.dt.float32
I32 = mybir.dt.int32
BF16 = mybir.dt.bfloat16

# Attention/AllToAll chunks: (batch, tok_start within batch, ntoks).
# Each chunk's O slice is exchanged with an AllToAll: core j receives all
# 16 heads' features for its SH-token shard of the chunk, then computes
# the full 2048-feature output projection for its own tokens only.
CHUNKS = [(t // S, t % S, 512) for t in range(0, TOK, 512)]
NCH = len(CHUNKS)
SH = 512 // NCORES          # tokens per core per chunk = 64
TPC = 4 * SH                # tokens per core per batch = 256

LAST_EXEC_NS = None
LAST_RES = None


def _build():
    nc = bass.Bass()
    TT = mybir.AluOpType
    AF = mybir.ActivationFunctionType

    # Host passes weight/rope tensors pre-rearranged partition-major so
    # the loads are single-span contiguous DMAs.
    xt = nc.declare_dram_parameter("xt", [DM, TOK], BF16, isOutput=False)
    wqkv = nc.declare_dram_parameter("wqkv", [128, 16, 3 * FC], BF16,
                                     isOutput=False)
    # full wo^T, partition-major (token-sharded output projection)
    wot = nc.declare_dram_parameter("wot", [128, 16, DM], BF16, isOutput=False)
    # rope tables: [128, sb, 2(q/k), 4(F00,F01,F10,F11), 64] bf16
    rope = nc.declare_dram_parameter("rope", [128, SB, 2, 4, 64], BF16,
                                     isOutput=False)
    # transposed output for this core's 2*TPC tokens: [of, tok]; host
    # transposes and scatters the token shards back
    out_ext = nc.declare_dram_parameter("out", [DM, B * TPC], F32,
                                        isOutput=True)

    xt_r = xt.rearrange("(c p) t -> p c t", p=128)        # [128, 16, 4096]

    with tile.TileContext(nc, num_cores=NCORES) as tc:
        from contextlib import ExitStack

        with ExitStack() as ctx:
            const = ctx.enter_context(tc.tile_pool(name="const", bufs=1))
            persist = ctx.enter_context(tc.tile_pool(name="persist", bufs=1))
            xt_pool = ctx.enter_context(tc.tile_pool(name="xtp", bufs=2))
            norm_pool = ctx.enter_context(tc.tile_pool(name="norm", bufs=2))
            et_pool = ctx.enter_context(tc.tile_pool(name="expp", bufs=2))
            den_pool = ctx.enter_context(tc.tile_pool(name="denp", bufs=2))
            ot_pool = ctx.enter_context(tc.tile_pool(name="otp", bufs=2))
            wo_in = ctx.enter_context(tc.tile_pool(name="woin", bufs=2))
            p_s3 = ctx.enter_context(
                tc.tile_pool(name="ps3", bufs=2, space="PSUM")
            )
            p_acc = ctx.enter_context(
                tc.tile_pool(name="pacc", bufs=4, space="PSUM")
            )
            dram = ctx.enter_context(tc.tile_pool(name="dram", bufs=1, space="DRAM"))

            # ---- constants (gpsimd queue; sync stays free for stores) ------
            w_sb = [
                const.tile([128, 4, 3 * FC], BF16, name=f"w_sb{g}")
                for g in range(4)
            ]
            nc.gpsimd.dma_start(out=w_sb[0], in_=wqkv[:, 0:4, :])

            def w_ap(ci):
                return w_sb[ci // 4][:, ci % 4]

            TOKC = 256
            xt_tiles = {}

            def load_xt(b, tci):
                # sync queue: the gpsimd queue stalls on collective
                # triggers (stream depth 1), which must never gate the
                # projection input stream
                t = xt_pool.tile([128, 16, TOKC], BF16, tag="xt")
                t0 = b * S + tci * TOKC
                nc.sync.dma_start(out=t, in_=xt_r[:, :, t0 : t0 + TOKC])
                xt_tiles[(b, tci)] = t

            # The first xt chunk is loaded in pieces, interleaved with the
            # weight groups, ordered so each arrives just before the first
            # matmul that needs it (block i needs xt cols i*128..; ci 4g
            # needs w group g).
            xt0 = xt_pool.tile([128, 16, TOKC], BF16, tag="xt")
            nc.gpsimd.dma_start(out=xt0[:, :, 0:128], in_=xt_r[:, :, 0:128])
            nc.gpsimd.dma_start(out=w_sb[1], in_=wqkv[:, 4:8, :])
            nc.gpsimd.dma_start(out=xt0[:, :, 128:256], in_=xt_r[:, :, 128:256])
            nc.gpsimd.dma_start(out=w_sb[2], in_=wqkv[:, 8:12, :])
            nc.gpsimd.dma_start(out=w_sb[3], in_=wqkv[:, 12:16, :])
            xt_tiles[(0, 0)] = xt0

            rope_sb = const.tile([128, SB, 2, 4, 64], BF16, name="rope_sb")
            nc.gpsimd.dma_start(out=rope_sb, in_=rope[:])
            ones_mat = const.tile([128, 128], BF16, name="ones_mat")
            nc.vector.memset(ones_mat, 1.0)
            ident = const.tile([128, 128], BF16, name="ident")
            make_identity(nc, ident)
            # warm the PE clock (HAM) while the first DMAs land
            pwu = p_acc.tile([128, 512], F32, tag="acc")
            for _ in range(80):
                nc.tensor.matmul(pwu[:, 0:128], lhsT=ones_mat, rhs=ones_mat,
                                 start=True, stop=True)

            # ---- per-batch persistent state (double-buffered) --------------
            QT = [persist.tile([128, HL, S], BF16, name=f"QT{b}") for b in range(B)]
            KT = [persist.tile([128, HL, S], BF16, name=f"KT{b}") for b in range(B)]
            V = [persist.tile([128, KB, FC], BF16, name=f"V{b}") for b in range(B)]

            # A2A buffers, one per batch: dim0 is split into 8 blocks of
            # FC rows.  ag_in block j = my FC features for core j's TPC
            # tokens (SH per chunk); ag_out block i = core i's FC
            # features for my TPC tokens.  One collective per batch: the
            # CC stream has a ~30us fixed cost per op regardless of size.
            # one A2A per (batch, local head): the hl=0 exchange can
            # trigger before the hl=1 attention finishes, and wo can
            # start accumulating on the first half while the second is
            # still in flight
            ag_in = {
                (b, hl): dram.tile([NCORES * 128, TPC], BF16,
                                   name=f"ag_in{b}_{hl}")
                for b in range(B) for hl in range(HL)
            }
            ag_out = {
                (b, hl): dram.tile([NCORES * 128, TPC], BF16,
                                   name=f"ag_out{b}_{hl}")
                for b in range(B) for hl in range(HL)
            }

            def a2a(b, hl):
                nc.gpsimd.collective_compute(
                    "AllToAll",
                    mybir.AluOpType.bypass,
                    replica_groups=[list(range(NCORES))],
                    ins=[ag_in[(b, hl)].opt()],
                    outs=[ag_out[(b, hl)].opt()],
                )

            # tiny warmup AllToAll: absorbs the CC stream's cold-start
            # latency (~40us on the first op) during proj0
            warm_in = dram.tile([8, 64], BF16, name="warm_in")
            warm_out = dram.tile([8, 64], BF16, name="warm_out")
            warm_sb = const.tile([8, 64], BF16, name="warm_sb")
            nc.vector.memset(warm_sb, 0.0)
            nc.sync.dma_start(out=warm_in, in_=warm_sb)
            nc.gpsimd.collective_compute(
                "AllToAll",
                mybir.AluOpType.bypass,
                replica_groups=[list(range(NCORES))],
                ins=[warm_in.opt()],
                outs=[warm_out.opt()],
            )


            # =================================================================
            # phase builders
            # =================================================================

            def proj_phase(b, post_tci=None, evict_on_act=True):
                """q/k/v projections + RMSNorm + RoPE + transposes for batch b.
                Writes QT/KT and V (bf16).  Transposes for token block i
                are interleaved into block i+1's matmul stream so the
                accumulating PSUM never stalls the PE; they land in the
                otherwise-idle third PSUM bank of block i's s3 tile.
                proj_tcis(b) returns the same work as 4 per-tci closures so
                the caller can interleave attention chunks between them."""
                for emit in proj_tcis(b, evict_on_act):
                    emit()
                    if post_tci is not None:
                        post_tci(emit.tci)

            def proj_tcis(b, evict_on_act=True):
                pending = []  # (accb, j, sb) awaiting eviction to QT/KT

                def emit_transpose(j, qr, accb, sb):
                    nc.tensor.transpose(
                        accb[:, j * 128 : (j + 1) * 128], qr[:, j], ident
                    )
                    pending.append((accb, j, sb))

                def emit_evict(accb, j, sb):
                    # plain proj phase: ACT has slack; interleaved with
                    # attention (exp hogs ACT's FIFO): use DVE instead
                    PT = KT[b] if j >= 2 else QT[b]
                    hl = j % 2
                    dst = PT[:, hl, sb * 128 : (sb + 1) * 128]
                    src = accb[:, j * 128 : (j + 1) * 128]
                    if evict_on_act:
                        nc.scalar.activation(out=dst, in_=src, func=AF.Copy)
                    else:
                        nc.vector.tensor_copy(out=dst, in_=src)

                tposes = []   # transposes awaiting emission

                def emit_tci(tci):
                    if (b, tci) not in xt_tiles:
                        load_xt(b, tci)
                    xt_sb = xt_tiles.pop((b, tci))
                    # prefetch next chunk
                    nxt = (b, tci + 1) if tci + 1 < S // TOKC else (b + 1, 0)
                    if nxt[0] < B and nxt not in xt_tiles:
                        load_xt(*nxt)
                    for tbl in range(TOKC // 128):
                        sb = tci * (TOKC // 128) + tbl   # seq block 0..15
                        s3 = p_s3.tile([128, 1024], F32, tag="s3")
                        pqA = s3[:, 0:384]
                        pqB = s3[:, 512:896]
                        for ci in range(16):
                            lhsT = xt_sb[:, ci, tbl * 128 : (tbl + 1) * 128]
                            nc.tensor.matmul(
                                pqA, lhsT=lhsT, rhs=w_ap(ci)[:, 0:384],
                                start=(ci == 0), stop=(ci == 15),
                            )
                            nc.tensor.matmul(
                                pqB, lhsT=lhsT, rhs=w_ap(ci)[:, 384:768],
                                start=(ci == 0), stop=(ci == 15),
                            )
                            # interleave previous block's transposes between
                            # accumulation steps (every 4th ci)
                            if ci % 4 == 3 and tposes:
                                emit_transpose(*tposes.pop(0))
                        while tposes:
                            emit_transpose(*tposes.pop(0))
                        while pending:
                            emit_evict(*pending.pop(0))

                        # free the psum banks quickly:
                        # qraw = [q_h0, q_h1, k_h0, k_h1] [128, 4, 128] fp32
                        qraw = norm_pool.tile([128, 4, 128], F32, tag="qraw")
                        if evict_on_act:
                            nc.scalar.activation(
                                out=qraw[:, 0:3], in_=pqA, func=AF.Copy
                            )
                            nc.scalar.activation(
                                out=qraw[:, 3], in_=s3[:, 512:640], func=AF.Copy
                            )
                            nc.scalar.activation(
                                out=V[b][:, sb, :], in_=s3[:, 640:896],
                                func=AF.Copy,
                            )
                        else:
                            nc.vector.tensor_copy(out=qraw[:, 0:3], in_=pqA)
                            nc.vector.tensor_copy(
                                out=qraw[:, 3], in_=s3[:, 512:640]
                            )
                            nc.vector.tensor_copy(
                                out=V[b][:, sb, :], in_=s3[:, 640:896]
                            )

                        # rms stats: rstd = rsqrt(mean(t^2)+eps), table-free
                        # Newton on DVE for (q_h0, q_h1, k_h0, k_h1)
                        sqs = norm_pool.tile([128, 4, 128], BF16, tag="sqs")
                        ssum = norm_pool.tile([128, 4], F32, tag="ssum")
                        nc.vector.tensor_tensor(
                            out=sqs, in0=qraw, in1=qraw, op=TT.mult
                        )
                        nc.vector.tensor_reduce(
                            out=ssum, in_=sqs, axis=mybir.AxisListType.X,
                            op=TT.add,
                        )
                        v_ = norm_pool.tile([128, 4], F32, tag="v_")
                        nc.vector.tensor_scalar(
                            out=v_, in0=ssum, scalar1=1.0 / DH, scalar2=EPS,
                            op0=TT.mult, op1=TT.add,
                        )
                        y = norm_pool.tile([128, 4], F32, tag="y")
                        t_ = norm_pool.tile([128, 4], F32, tag="t_")
                        u_ = norm_pool.tile([128, 4], F32, tag="u_")
                        # seed: y0 = bits(0x5f3759df - (bits(v) >> 1))
                        nc.vector.tensor_scalar(
                            out=y.bitcast(I32), in0=v_.bitcast(I32),
                            scalar1=1, scalar2=None,
                            op0=TT.logical_shift_right,
                        )
                        nc.vector.tensor_scalar(
                            out=y.bitcast(I32), in0=y.bitcast(I32),
                            scalar1=-1, scalar2=0x5F3759DF,
                            op0=TT.mult, op1=TT.add,
                        )
                        for it in range(2):  # Newton: y *= 1.5 - 0.5 v y^2
                            nc.vector.tensor_tensor(
                                out=t_, in0=y, in1=y, op=TT.mult
                            )
                            nc.vector.tensor_tensor(
                                out=t_, in0=t_, in1=v_, op=TT.mult
                            )
                            nc.vector.tensor_scalar(
                                out=u_, in0=t_, scalar1=-0.5, scalar2=1.5,
                                op0=TT.mult, op1=TT.add,
                            )
                            nc.vector.tensor_tensor(
                                out=y, in0=y, in1=u_, op=TT.mult
                            )

                        # apply norm in place (DVE; y broadcast over head_dim)
                        nc.vector.tensor_tensor(
                            out=qraw, in0=qraw,
                            in1=y[:, :, None].to_broadcast((128, 4, 128)),
                            op=TT.mult,
                        )
                        # rope: fp32 math, single bf16 rounding at the output
                        qr = norm_pool.tile([128, 4, 128], BF16, tag="qr")
                        qp = norm_pool.tile([128, 2, 2, 64], F32, tag="qp")
                        qn4 = qraw.rearrange("p (k h) d -> p k h d", k=2)
                        qr4 = qr.rearrange("p (k h) d -> p k h d", k=2)
                        lo = qn4[:, :, :, 0:64]
                        hi = qn4[:, :, :, 64:128]

                        def f(r):
                            return rope_sb[:, sb, :, None, r, :].to_broadcast(
                                (128, 2, 2, 64)
                            )

                        tmp = norm_pool.tile([128, 2, 2, 64], F32, tag="tmp")
                        nc.vector.tensor_tensor(
                            out=qp, in0=lo, in1=f(0), op=TT.mult
                        )
                        nc.vector.tensor_tensor(
                            out=tmp, in0=hi, in1=f(1), op=TT.mult
                        )
                        nc.vector.tensor_tensor(
                            out=qr4[:, :, :, 0:64], in0=qp, in1=tmp, op=TT.add
                        )
                        nc.vector.tensor_tensor(
                            out=qp, in0=lo, in1=f(2), op=TT.mult
                        )
                        nc.vector.tensor_tensor(
                            out=tmp, in0=hi, in1=f(3), op=TT.mult
                        )
                        nc.vector.tensor_tensor(
                            out=qr4[:, :, :, 64:128], in0=qp, in1=tmp, op=TT.add
                        )
                        # transpose quad for this block goes into an acc
                        # tile (bitcast to bf16: 4 x [128,128] in half a
                        # bank) so the s3 tile is released by the qraw/V
                        # evictions alone
                        acc = p_acc.tile([128, 512], F32, tag="acc")
                        accb = acc.bitcast(BF16)   # [128, 1024] bf16
                        for j in range(4):
                            tposes.append((j, qr, accb, sb))
                    # flush deferred transposes before handing the s3/acc
                    # rotation to whatever the caller interleaves next
                    while tposes:
                        emit_transpose(*tposes.pop(0))
                    while pending:
                        emit_evict(*pending.pop(0))

                emits = []
                for tci in range(S // TOKC):           # 4 chunks of 512 tokens
                    e = (lambda t: (lambda: emit_tci(t)))(tci)
                    e.tci = tci
                    emits.append(e)
                return emits

            def attn_chunk(ch):
                """attention for AG chunk ch (one batch, one q-range) +
                its AllGather."""
                b, q0, nt = CHUNKS[ch]
                for hl in range(HL):
                    po = p_acc.tile([128, 512], F32, tag="acc")
                    pden = p_acc.tile([128, 512], F32, tag="acc")
                    et = et_pool.tile([128, KB, 512], BF16, tag="et")
                    # den quad-sums: et summed over kb in groups of 4 on
                    # DVE (bf16 pair tree), so the softmax-denominator
                    # ones-matmul runs 4x fewer columns on the PE.
                    et5 = et.rearrange("p (k a b) q -> p k a b q", a=2, b=2)
                    den4 = [
                        den_pool.tile([128, 512], BF16, tag=f"den4_{k}",
                                      name=f"den4_{k}")
                        for k in range(4)
                    ]
                    # kb groups of 2; a small first group primes the
                    # PE<->ACT pipeline (PV can start after a 1-block exp)
                    groups = [(0, 1)] + [
                        (g, min(g + 2, KB)) for g in range(1, KB, 2)
                    ]

                    def scores(g0, g1):
                        s3 = p_s3.tile([128, 1024], F32, tag="s3")
                        for kb in range(g0, g1):
                            nc.tensor.matmul(
                                s3[:, (kb - g0) * 512 : (kb - g0) * 512 + nt],
                                lhsT=KT[b][:, hl, kb * 128 : (kb + 1) * 128],
                                rhs=QT[b][:, hl, q0 : q0 + nt],
                                start=True, stop=True,
                            )
                        return s3

                    def exp(s3, g0, g1):
                        # one ACT call over the whole group (cross-bank read)
                        nc.scalar.activation(
                            out=et[:, g0:g1, 0:nt],
                            in_=s3[:, 0 : (g1 - g0) * 512].rearrange(
                                "p (k n) -> p k n", k=g1 - g0
                            )[:, :, 0:nt],
                            func=AF.Exp,
                        )

                    def den_tree(k):
                        # quad k covers kb 4k..4k+3 (quad 3 only 12..14:
                        # kb15 streams straight into the pden matmul so
                        # the chunk tail never waits on the DVE)
                        ts1 = den_pool.tile([128, 2, 512], BF16, tag="ts1",
                                            name="ts1")
                        if k < 3:
                            nc.vector.tensor_tensor(
                                out=ts1[:, :, 0:nt],
                                in0=et5[:, k, :, 0, 0:nt],
                                in1=et5[:, k, :, 1, 0:nt],
                                op=TT.add,
                            )
                            nc.vector.tensor_tensor(
                                out=den4[k][:, 0:nt],
                                in0=ts1[:, 0, 0:nt],
                                in1=ts1[:, 1, 0:nt],
                                op=TT.add,
                            )
                        else:
                            nc.vector.tensor_tensor(
                                out=ts1[:, 0, 0:nt],
                                in0=et[:, 12, 0:nt],
                                in1=et[:, 13, 0:nt],
                                op=TT.add,
                            )
                            nc.vector.tensor_tensor(
                                out=den4[k][:, 0:nt],
                                in0=ts1[:, 0, 0:nt],
                                in1=et[:, 14, 0:nt],
                                op=TT.add,
                            )

                    def den_mm(k):
                        nc.tensor.matmul(
                            pden[:, 0:nt], lhsT=ones_mat,
                            rhs=den4[k][:, 0:nt],
                            start=(k == 0), stop=False,
                        )

                    def den_mm_last():
                        nc.tensor.matmul(
                            pden[:, 0:nt], lhsT=ones_mat,
                            rhs=et[:, 15, 0:nt],
                            start=False, stop=True,
                        )

                    def pv(g0, g1):
                        for kb in range(g0, g1):
                            nc.tensor.matmul(
                                po[:, 0:nt],
                                lhsT=V[b][:, kb, hl * 128 : (hl + 1) * 128],
                                rhs=et[:, kb, 0:nt],
                                start=(kb == 0), stop=(kb == KB - 1),
                            )

                    # quad k's tree can be emitted once exp covers its
                    # kbs; its matmul is deferred one group so the DVE
                    # result is ready when the PE reaches it.
                    tree_at = {5: 0, 9: 1, 13: 2, 15: 3}
                    prev = None
                    mm_q = []
                    for (g0, g1) in groups:
                        s3 = scores(g0, g1)
                        exp(s3, g0, g1)
                        if prev is not None:
                            pv(*prev)
                        if mm_q:
                            den_mm(mm_q.pop(0))
                        if g1 in tree_at:
                            den_tree(tree_at[g1])
                            mm_q.append(tree_at[g1])
                        prev = (g0, g1)
                    pv(*prev)
                    while mm_q:
                        den_mm(mm_q.pop(0))
                    den_mm_last()

                    # evict po/pden to SBUF fast (frees the acc banks),
                    # then normalize from SBUF off the PE-critical path
                    po_s = ot_pool.tile([128, 512], F32, tag="po_s")
                    den_s = ot_pool.tile([128, 512], F32, tag="den_s")
                    nc.vector.tensor_copy(out=den_s[:, 0:nt], in_=pden[:, 0:nt])
                    nc.vector.tensor_copy(out=po_s[:, 0:nt], in_=po[:, 0:nt])
                    if os.environ.get("ATTN_FAST_RECIP", "0") == "1":
                        recip = ot_pool.tile([128, 512], F32, tag="recip")
                        nc.vector.reciprocal_approx_fast(
                            out=recip[:, 0:nt], in_=den_s[:, 0:nt]
                        )
                    else:
                        recip = ot_pool.tile([128, 512], F32, tag="recip")
                        nc.vector.reciprocal(
                            out=recip[:, 0:nt], in_=den_s[:, 0:nt]
                        )
                    ot = ot_pool.tile([128, 512], BF16, tag="ot")
                    nc.vector.tensor_tensor(
                        out=ot[:, 0:nt], in0=po_s[:, 0:nt],
                        in1=recip[:, 0:nt], op=TT.mult,
                    )
                    # scatter into the A2A staging layout: dest block j
                    # gets this head's rows for core j's SH-token shard,
                    # at this chunk's SH-column offset within the batch
                    kk = ch % 4
                    nc.sync.dma_start(
                        out=ag_in[(b, hl)].rearrange(
                            "(j f) (k u) -> f j k u", j=NCORES, k=4
                        )[:, :, kk],
                        in_=ot[:, 0:nt].rearrange("p (j u) -> p j u", j=NCORES),
                    )

            otf_b = {}

            def wo_load(b, hl):
                """land a (batch, local-head) AllToAll result (8 cores x
                128 features for my TPC tokens).  sync queue: a wo_load
                waits on its AllToAll, and the gpsimd queue (collective
                triggers) must not stall on it."""
                t = wo_in.tile([128, NCORES, TPC], BF16, tag=f"otf{hl}",
                               name=f"otf_b{b}_{hl}")
                ag_r = ag_out[(b, hl)].rearrange("(c p) u -> p c u", p=128)
                nc.sync.dma_start(out=t, in_=ag_r)
                otf_b[(b, hl)] = t

            wot_tiles = {}

            def wot_load(key, e):
                """stream an of-eighth (256 output features) of wo^T into
                an xt-pool tile (free after proj1).  gpsimd queue: idle
                during the wo drain, keeping the sync queue for ot/out."""
                t = xt_pool.tile([128, 16, TOKC], BF16, tag="xt")
                nc.gpsimd.dma_start(
                    out=t, in_=wot[:, :, e * 256 : (e + 1) * 256]
                )
                wot_tiles[key] = t

            def wo_mm8(b, e):
                """output projection for one (batch, of-eighth): all of my
                TPC tokens of batch b against 256 output features.  The
                contraction runs hl=0 feature blocks first so it can start
                as soon as the first A2A half lands."""
                w = wot_tiles.pop((b, e))
                pw = p_acc.tile([128, 512], F32, tag="acc")
                for ofb in range(2):
                    for m, ci in enumerate(
                        [2 * c + hl for hl in range(HL) for c in range(8)]
                    ):
                        nc.tensor.matmul(
                            pw[:, ofb * TPC : (ofb + 1) * TPC],
                            lhsT=w[:, ci, ofb * 128 : (ofb + 1) * 128],
                            rhs=otf_b[(b, ci % 2)][:, ci // 2, :],
                            start=(m == 0), stop=(m == 15),
                        )
                osb = ot_pool.tile([128, 512], F32, tag="osb")
                nc.vector.tensor_copy(out=osb, in_=pw)
                nc.sync.dma_start(
                    out=out_ext[
                        e * 256 : (e + 1) * 256, b * TPC : (b + 1) * TPC
                    ].rearrange("(a p) t -> p a t", a=2),
                    in_=osb.rearrange("p (a t) -> p a t", a=2),
                )

            # ---- emission order (controls per-engine instruction order) ----
            # proj0 -> attn0 -> proj1 -> attn1: batch 0's AllGathers run
            # during attn0/proj1, batch 1's during attn1, so the comm stream
            # spans most of the kernel and only the last small chunk's AG is
            # exposed.  wo for chunk c is scheduled well after its AllGather
            # lands (otf prefetched one hook earlier; never before the AG's
            # trigger is emitted, else the gpsimd queue deadlocks).
            # proj0, then batch-0 attention chunks interleaved with proj1's
            # token chunks: the AllGather stream starts ~40us after proj0
            # and stays busy through proj1, and batch-1 attention (whose 6
            # AGs are the serial tail) starts ~80us earlier than running
            # attn0 and proj1 back-to-back would allow.  wo for chunk c is
            # scheduled 2+ chunks after its AllGather is triggered.
            proj_phase(0)
            p1 = proj_tcis(1)
            attn_chunk(0); p1[0](); p1[1]()
            attn_chunk(1); p1[2](); p1[3]()
            attn_chunk(2); p1[4](); p1[5]()
            attn_chunk(3); a2a(0, 0); a2a(0, 1); p1[6](); p1[7]()
            # batch-1 attention back-to-back; batch-0's A2As (~30us CC
            # ops) land under it.  All wo work drains after the last
            # chunk so batch-1's A2As trigger as early as possible; the
            # hl=1 half hides under wo's hl=0 contraction steps.
            attn_chunk(4)
            wo_load(0, 0); wo_load(0, 1)
            wot_load((0, 0), 0); wot_load((0, 1), 1)
            attn_chunk(5)
            attn_chunk(6)
            attn_chunk(7); a2a(1, 0); a2a(1, 1)
            for e in range(8):
                wo_mm8(0, e)
                if e < 6:
                    wot_load((0, e + 2), e + 2)
                else:
                    wot_load((1, e - 6), e - 6)
                if e == 5:
                    # sync reaches here after b0's e<=5 out stores; the
                    # A2A(1,*) waits no longer gate anything behind them
                    wo_load(1, 0); wo_load(1, 1)
            for e in range(8):
                wo_mm8(1, e)
                if e < 6:
                    wot_load((1, e + 2), e + 2)

    return nc


def _prep_inputs(x, rope_emb, wq, wk, wv, wo, q_norm_w, k_norm_w):
    """Host-side shard prep: per-core input maps."""
    bf = ml_dtypes.bfloat16
    X = np.ascontiguousarray(x.reshape(TOK, DM))
    xt = np.ascontiguousarray(X.T).astype(bf)  # [DM, TOK]

    gamma = 1.0 / np.sqrt(DH)
    qw = np.asarray(q_norm_w, np.float32)
    kw = np.asarray(k_norm_w, np.float32)
    fr = np.asarray(rope_emb, np.float32)[:, 0]  # [S, 64, 2, 2]

    def rope_pack(w, scale):
        # F[r] for r=(i,l): out[i*64+j] += F[i,l][s,j] * t[l*64+j], t = norm*w
        F = np.empty((S, 4, 64), np.float32)
        F[:, 0] = fr[:, :, 0, 0] * w[None, :64] * scale
        F[:, 1] = fr[:, :, 0, 1] * w[None, 64:] * scale
        F[:, 2] = fr[:, :, 1, 0] * w[None, :64] * scale
        F[:, 3] = fr[:, :, 1, 1] * w[None, 64:] * scale
        return F

    rope_all = np.stack([rope_pack(qw, gamma), rope_pack(kw, 1.0)], axis=1)
    # [S, 2, 4, 64] -> partition-major [128, SB, 2, 4, 64]
    rope_pm = np.ascontiguousarray(
        rope_all.reshape(SB, 128, 2, 4, 64).transpose(1, 0, 2, 3, 4)
    ).astype(bf)

    def pmajor(a):
        # [DM, F] -> [128, 16, F] with dm = c*128 + p
        return np.ascontiguousarray(
            a.reshape(16, 128, a.shape[1]).transpose(1, 0, 2)
        )

    wot_full = pmajor(np.ascontiguousarray(wo.T).astype(bf))  # [128,16,2048]
    in_maps = []
    for c in range(NCORES):
        rows = slice(c * FC, (c + 1) * FC)
        wqkv = np.concatenate(
            [wq[rows].T, wk[rows].T, wv[rows].T], axis=1
        ).astype(bf)  # [DM, 768]
        in_maps.append(
            {
                "xt": xt,
                "wqkv": pmajor(wqkv),
                "wot": wot_full,
                "rope": rope_pm,
            }
        )
    return in_maps


_CACHE = {}


def kernel(x, rope_emb, wq, wk, wv, wo, q_norm_w, k_norm_w):
    global LAST_EXEC_NS, LAST_RES
    x = np.asarray(x, np.float32)
    rope_emb = np.asarray(rope_emb, np.float32)
    wq = np.asarray(wq, np.float32)
    wk = np.asarray(wk, np.float32)
    wv = np.asarray(wv, np.float32)
    wo = np.asarray(wo, np.float32)
    q_norm_w = np.asarray(q_norm_w, np.float32)
    k_norm_w = np.asarray(k_norm_w, np.float32)
    _patch_tile_drain()
    _patch_ldw_opt()
    _register_ntff_hook()
    from concourse.bass_utils import run_bass_kernel_spmd

    if "nc" not in _CACHE:
        nc = _build()
        if os.environ.get("ATTN_STRIP_INCS", "0") == "1":
            _strip_redundant_incs(nc)
        _legalize_waits(nc)
        _CACHE["nc"] = nc
    nc = _CACHE["nc"]

    in_maps = _prep_inputs(x, rope_emb, wq, wk, wv, wo, q_norm_w, k_norm_w)
    trace = os.environ.get("ATTN_TRACE", "0") == "1"
    res = run_bass_kernel_spmd(
        nc, in_maps, core_ids=list(range(NCORES)), trace=trace
    )
    LAST_EXEC_NS = res.exec_time_ns
    LAST_RES = res

    # out_ext is [DM, B*TPC] per core: all 2048 output features for the
    # core's token shards (SH consecutive tokens per 512-token chunk).
    out = np.empty((B, S, DM), np.float32)
    for c in range(NCORES):
        oc = res.results[c]["out"]  # [DM, B*TPC]
        for b in range(B):
            for k in range(4):       # chunk k within batch
                sl = oc[:, b * TPC + k * SH : b * TPC + (k + 1) * SH]
                q0 = k * 512 + c * SH
                out[b, q0 : q0 + SH, :] = sl.T
    return np.ascontiguousarray(out)



# revision 52
# speedup vs baseline: 1.0105x; 1.0105x over previous
"""Distributed Bass kernel for nn_Attention_16509854286348.

Strategy (8 NeuronCores, tensor-parallel over heads):
  - Each core owns 2 of the 16 heads: it computes q/k/v projections for
    its 256 output features from the (replicated) input x, applies
    RMSNorm + RoPE (norm weights and the 1/sqrt(dh) attention scale are
    folded into the rope factor tables on the host), runs attention for
    its (batch, head) pairs, and produces O^T [256, tok] slices.
  - The O^T slices are AllGathered in per-(batch, q-tile) chunks.
    Phase order proj0 -> attn0 -> proj1 -> attn1 (per-batch q/k/v state
    is double-buffered) starts the AllGather stream ~120us earlier than
    attention-at-the-end would; batch 0's wo chunks run inside proj1,
    batch 1's lag their chunk by 2 so their AllGathers have landed; the
    final chunks are small to shrink the exposed tail.
  - After each AllGather lands, the core computes a disjoint 256-row
    slice of the output projection, transposed ([f, tok]); the host
    transposes and concatenates the 8 slices.

Numerics: bf16 matmul operands with fp32 PSUM accumulation; the q/k
path keeps fp32 math through RMSNorm and RoPE and rounds to bf16 once
(at the rope output); softmax statistics fp32.  Scores are O(1) by
construction (RMS-normed q/k), so softmax skips the max subtraction.

Engine layout:
  - PE: projections as 384/384-column matmul pairs (the 128-column
    LDWEIGHTS for the next stationary operand hides under both), scores
    / PV / denominator matmuls at N=512, PE-transposes for the q/k
    blocks, output projection at N=512.
  - ACT: PSUM evictions in the projection phase; Exp in 3-block groups
    ([128,1536] reads across PSUM banks) to amortize the ~293ns fixed
    cost per activate.
  - DVE: rms stats (table-free Newton rsqrt), norm scale, rope,
    po/pden evictions, softmax normalize (reciprocal_approx_fast).
  - PSUM: 2 pools exactly filling 8 banks: "s3" [128,1536]x2 (proj
    accum + scores + exp source), "acc" [128,512]x2 (transpose quads
    in proj, po/pden in attention, wo accum).
"""

import os
import sys
import types

import numpy as np
import ml_dtypes

import concourse.bass as bass
import concourse.mybir as mybir
import concourse.tile as tile
from concourse.masks import make_identity

# ---------------------------------------------------------------------------
# Environment workarounds
# ---------------------------------------------------------------------------


def _patch_tile_drain():
    """walrus in this image rejects >1 sem wait on the TileContext exit
    drain ("Too many sync wait commands"); split the waits into
    individual single-wait nops on the sync engine."""
    import bass_rust
    from concourse import tile as _tile
    from concourse.vector_clock import ScopedClock

    if getattr(_tile.TileContext, "_drain_patched", False):
        return

    def _drain_and_barrier(self, tick_clock, wait_clock):
        nc = self.nc
        drain_inst = nc.sync.drain()
        wait_clock.add_sem_waits(
            drain_inst.ins, ScopedClock({None: tick_clock.global_clock})
        )
        si = drain_inst.ins.sync_info
        if si is not None and len(si.on_wait) > 1:
            waits = list(si.on_wait)
            updates = list(si.on_update)
            drain_inst.ins.sync_info = bass_rust.SyncInfo(
                on_wait=[], on_update=updates
            )
            for w in waits:
                n = nc.sync.nop(nofuse=True)
                n.ins.sync_info = bass_rust.SyncInfo(on_wait=[w], on_update=[])
        nc.all_engine_barrier()
        assert self.sems is not None
        popped = nc._tile_sem_poison_stack.pop()
        assert popped is self._sem_poison
        nc.clear_and_free_semaphores(list(self.sems.allocated().values()))
        nc.all_engine_barrier()

    _tile.TileContext._drain_and_barrier = _drain_and_barrier
    _tile.TileContext._drain_patched = True


def _legalize_waits(nc, max_waits=1):
    """This image's walrus rejects instructions with more than one sync
    wait ("Too many sync wait commands").  Hoist excess waits onto
    dedicated single-wait nops inserted just before the instruction on
    the same engine — semantically identical, since the engine stalls on
    the nops first."""
    import bass_rust

    counter = [0]
    for f in nc.m.functions:
        for bb in f.blocks:
            changed = False
            newlist = []
            for ins in bb.instructions:
                si = ins.sync_info
                if si is not None and len(si.on_wait) > max_waits:
                    waits = list(si.on_wait)
                    updates = list(si.on_update)
                    for w in waits[:-max_waits]:
                        counter[0] += 1
                        nop = mybir.InstNoOp(
                            name=f"LGW-{counter[0]}", ins=[], outs=[]
                        )
                        nop.engine = ins.engine
                        nop.sync_info = bass_rust.SyncInfo(
                            on_wait=[w], on_update=[]
                        )
                        newlist.append(nop)
                    ins.sync_info = bass_rust.SyncInfo(
                        on_wait=waits[-max_waits:], on_update=updates
                    )
                    changed = True
                newlist.append(ins)
            if changed:
                bb.instructions = newlist


def _patch_ldw_opt():
    """Turn on walrus's LDWEIGHTS optimization (hardcoded off in this
    image's bass_utils): rewrites the flag in the walrus_driver argv."""
    if os.environ.get("ATTN_LDW_OPT", "0") != "1":
        return
    import concourse.bass_utils as bu

    if getattr(bu, "_ldw_opt_patched", False):
        return
    orig = bu.run_command

    def run_command_ldw(argv, **kwargs):
        if isinstance(argv, list):
            argv = [
                "--enable-ldw-opt=true" if a == "--enable-ldw-opt=false" else a
                for a in argv
            ]
        return orig(argv, **kwargs)

    bu.run_command = run_command_ldw
    bu._ldw_opt_patched = True


def _strip_redundant_incs(nc, engine_insts=("InstMatmult",)):
    """Remove per-instruction semaphore increments that nothing waits on.

    The Tile framework increments a per-engine counting semaphore on
    every instruction; on the PE a serialized EVT_SEM write costs ~26ns
    per matmul (~6% of a N=512 matmul).  Engines complete instructions
    in program order, so a wait for "first k instructions done" is
    equivalent to "instruction #k done".  For each semaphore whose
    updaters are all same-engine instructions of the allowed types, keep
    increments only at the waited-on positions and renumber the wait
    thresholds to their rank.
    """
    import bass_rust

    all_ins = []
    for f in nc.m.functions:
        for bb in f.blocks:
            all_ins.extend(bb.instructions)

    updaters = {}   # sem id -> [(ins, engine, typename)]
    waits = {}      # sem id -> [(ins, SyncWait)]
    bad = set()     # sems we must not touch
    for ins in all_ins:
        si = ins.sync_info
        if si is None:
            continue
        for u in si.on_update:
            if (
                u.sync_type == "semaphore"
                and u.update_mode == "sem-inc"
                and u.update_value == 1
            ):
                updaters.setdefault(u.id, []).append(
                    (ins, ins.engine, type(ins).__name__)
                )
            else:
                bad.add(u.id)
        for w in si.on_wait:
            if w.sync_type == "semaphore":
                if w.wait_mode != "sem-ge-imm":
                    bad.add(w.id)
                else:
                    waits.setdefault(w.id, []).append((ins, w))

    for sem_id, ups in updaters.items():
        if sem_id in bad:
            continue
        engines = {e for _, e, _ in ups}
        types = {t for _, _, t in ups}
        if len(engines) != 1 or not types.issubset(set(engine_insts)):
            continue
        ks = sorted({w.wait_value for _, w in waits.get(sem_id, [])})
        if ks and (ks[0] < 1 or ks[-1] > len(ups)):
            continue  # threshold out of range: scheme not understood; skip
        rank = {k: i + 1 for i, k in enumerate(ks)}
        keep = set(ks)
        for pos, (ins, _, _) in enumerate(ups, start=1):
            if pos in keep:
                continue
            si = ins.sync_info
            ups_new = [
                u
                for u in si.on_update
                if not (u.sync_type == "semaphore" and u.id == sem_id)
            ]
            ins.sync_info = bass_rust.SyncInfo(
                on_wait=list(si.on_wait), on_update=ups_new
            )
        seen = set()
        for ins, w in waits.get(sem_id, []):
            if id(ins) in seen:
                continue
            seen.add(id(ins))
            si = ins.sync_info
            new_waits = []
            for ww in si.on_wait:
                if ww.sync_type == "semaphore" and ww.id == sem_id:
                    new_waits.append(
                        bass_rust.SyncWait(
                            sync_type=ww.sync_type,
                            id=ww.id,
                            ant_name=ww.ant_name,
                            wait_mode=ww.wait_mode,
                            wait_value=rank[ww.wait_value],
                            wait_reg=ww.wait_reg,
                        )
                    )
                else:
                    new_waits.append(ww)
            ins.sync_info = bass_rust.SyncInfo(
                on_wait=new_waits, on_update=list(si.on_update)
            )


def _register_ntff_hook():
    """The image's antenv package lacks axon_hooks; supply it so
    run_bass_kernel_spmd(trace=True) can profile under axon."""
    if "antenv.axon_hooks" in sys.modules:
        return
    import antenv

    mod = types.ModuleType("antenv.axon_hooks")
    mod._hook = None

    def set_axon_ntff_profile_hook(h):
        mod._hook = h

    def get_axon_ntff_profile_hook():
        return mod._hook

    mod.set_axon_ntff_profile_hook = set_axon_ntff_profile_hook
    mod.get_axon_ntff_profile_hook = get_axon_ntff_profile_hook
    sys.modules["antenv.axon_hooks"] = mod
    antenv.axon_hooks = mod
    try:
        from trn_agent_boot.trn_boot import _ntff_profile_via_ctypes

        mod.set_axon_ntff_profile_hook(
            _ntff_profile_via_ctypes("/opt/axon/libaxon_pjrt.so")
        )
    except Exception:
        pass


# ---------------------------------------------------------------------------
# Problem constants (hardcoded per spec)
# ---------------------------------------------------------------------------

B, S, DM = 2, 2048, 2048
H, DH = 16, 128
EPS = 1e-6
NCORES = 8
HL = H // NCORES            # heads per core = 2
FC = HL * DH                # feature slice per core = 256
TOK = B * S                 # 4096
SB = S // 128               # 16 seq blocks per batch
KB = S // 128               # 16 key blocks per batch

F32 = mybir.dt.float32
I32 = mybir.dt.int32
BF16 = mybir.dt.bfloat16

# Attention/AllToAll chunks: (batch, tok_start within batch, ntoks).
# Each chunk's O slice is exchanged with an AllToAll: core j receives all
# 16 heads' features for its SH-token shard of the chunk, then computes
# the full 2048-feature output projection for its own tokens only.
CHUNKS = [(t // S, t % S, 512) for t in range(0, TOK, 512)]
NCH = len(CHUNKS)
SH = 512 // NCORES          # tokens per core per chunk = 64
TPC = 4 * SH                # tokens per core per batch = 256

LAST_EXEC_NS = None
LAST_RES = None


def _build():
    nc = bass.Bass()
    TT = mybir.AluOpType
    AF = mybir.ActivationFunctionType

    # Host passes weight/rope tensors pre-rearranged partition-major so
    # the loads are single-span contiguous DMAs.
    xt = nc.declare_dram_parameter("xt", [DM, TOK], BF16, isOutput=False)
    wqkv = nc.declare_dram_parameter("wqkv", [128, 16, 3 * FC], BF16,
                                     isOutput=False)
    # full wo^T, partition-major (token-sharded output projection)
    wot = nc.declare_dram_parameter("wot", [128, 16, DM], BF16, isOutput=False)
    # rope tables: [128, sb, 2(q/k), 4(F00,F01,F10,F11), 64] bf16
    rope = nc.declare_dram_parameter("rope", [128, SB, 2, 4, 64], BF16,
                                     isOutput=False)
    # transposed output for this core's 2*TPC tokens: [of, tok]; host
    # transposes and scatters the token shards back
    out_ext = nc.declare_dram_parameter("out", [DM, B * TPC], F32,
                                        isOutput=True)

    xt_r = xt.rearrange("(c p) t -> p c t", p=128)        # [128, 16, 4096]

    with tile.TileContext(nc, num_cores=NCORES) as tc:
        from contextlib import ExitStack

        with ExitStack() as ctx:
            const = ctx.enter_context(tc.tile_pool(name="const", bufs=1))
            persist = ctx.enter_context(tc.tile_pool(name="persist", bufs=1))
            xt_pool = ctx.enter_context(tc.tile_pool(name="xtp", bufs=2))
            norm_pool = ctx.enter_context(tc.tile_pool(name="norm", bufs=2))
            et_pool = ctx.enter_context(tc.tile_pool(name="expp", bufs=2))
            den_pool = ctx.enter_context(tc.tile_pool(name="denp", bufs=2))
            ot_pool = ctx.enter_context(tc.tile_pool(name="otp", bufs=2))
            wo_in = ctx.enter_context(tc.tile_pool(name="woin", bufs=2))
            p_s3 = ctx.enter_context(
                tc.tile_pool(name="ps3", bufs=2, space="PSUM")
            )
            p_acc = ctx.enter_context(
                tc.tile_pool(name="pacc", bufs=4, space="PSUM")
            )
            dram = ctx.enter_context(tc.tile_pool(name="dram", bufs=1, space="DRAM"))

            # ---- constants (gpsimd queue; sync stays free for stores) ------
            w_sb = [
                const.tile([128, 4, 3 * FC], BF16, name=f"w_sb{g}")
                for g in range(4)
            ]
            nc.gpsimd.dma_start(out=w_sb[0], in_=wqkv[:, 0:4, :])

            def w_ap(ci):
                return w_sb[ci // 4][:, ci % 4]

            TOKC = 256
            xt_tiles = {}

            def load_xt(b, tci):
                # sync queue: the gpsimd queue stalls on collective
                # triggers (stream depth 1), which must never gate the
                # projection input stream
                t = xt_pool.tile([128, 16, TOKC], BF16, tag="xt")
                t0 = b * S + tci * TOKC
                nc.sync.dma_start(out=t, in_=xt_r[:, :, t0 : t0 + TOKC])
                xt_tiles[(b, tci)] = t

            # The first xt chunk is loaded in pieces, interleaved with the
            # weight groups, ordered so each arrives just before the first
            # matmul that needs it (block i needs xt cols i*128..; ci 4g
            # needs w group g).
            xt0 = xt_pool.tile([128, 16, TOKC], BF16, tag="xt")
            nc.gpsimd.dma_start(out=xt0[:, :, 0:128], in_=xt_r[:, :, 0:128])
            nc.gpsimd.dma_start(out=w_sb[1], in_=wqkv[:, 4:8, :])
            nc.gpsimd.dma_start(out=xt0[:, :, 128:256], in_=xt_r[:, :, 128:256])
            nc.gpsimd.dma_start(out=w_sb[2], in_=wqkv[:, 8:12, :])
            nc.gpsimd.dma_start(out=w_sb[3], in_=wqkv[:, 12:16, :])
            xt_tiles[(0, 0)] = xt0

            rope_sb = const.tile([128, SB, 2, 4, 64], BF16, name="rope_sb")
            nc.gpsimd.dma_start(out=rope_sb, in_=rope[:])
            ones_mat = const.tile([128, 128], BF16, name="ones_mat")
            nc.vector.memset(ones_mat, 1.0)
            ident = const.tile([128, 128], BF16, name="ident")
            make_identity(nc, ident)
            # warm the PE clock (HAM) while the first DMAs land
            pwu = p_acc.tile([128, 512], F32, tag="acc")
            for _ in range(80):
                nc.tensor.matmul(pwu[:, 0:128], lhsT=ones_mat, rhs=ones_mat,
                                 start=True, stop=True)

            # ---- per-batch persistent state (double-buffered) --------------
            QT = [persist.tile([128, HL, S], BF16, name=f"QT{b}") for b in range(B)]
            KT = [persist.tile([128, HL, S], BF16, name=f"KT{b}") for b in range(B)]
            V = [persist.tile([128, KB, FC], BF16, name=f"V{b}") for b in range(B)]

            # A2A buffers, one per batch: dim0 is split into 8 blocks of
            # FC rows.  ag_in block j = my FC features for core j's TPC
            # tokens (SH per chunk); ag_out block i = core i's FC
            # features for my TPC tokens.  One collective per batch: the
            # CC stream has a ~30us fixed cost per op regardless of size.
            # one A2A per (batch, local head): the hl=0 exchange can
            # trigger before the hl=1 attention finishes, and wo can
            # start accumulating on the first half while the second is
            # still in flight
            ag_in = {
                (b, hl): dram.tile([NCORES * 128, TPC], BF16,
                                   name=f"ag_in{b}_{hl}")
                for b in range(B) for hl in range(HL)
            }
            ag_out = {
                (b, hl): dram.tile([NCORES * 128, TPC], BF16,
                                   name=f"ag_out{b}_{hl}")
                for b in range(B) for hl in range(HL)
            }

            def a2a(b, hl):
                nc.gpsimd.collective_compute(
                    "AllToAll",
                    mybir.AluOpType.bypass,
                    replica_groups=[list(range(NCORES))],
                    ins=[ag_in[(b, hl)].opt()],
                    outs=[ag_out[(b, hl)].opt()],
                )

            def a2a_warm():
                # tiny op to keep the CC stream warm: the first collective
                # after an idle period pays a ~25us ramp that a queued
                # follow-up does not
                nc.gpsimd.collective_compute(
                    "AllToAll",
                    mybir.AluOpType.bypass,
                    replica_groups=[list(range(NCORES))],
                    ins=[warm_in.opt()],
                    outs=[warm_out.opt()],
                )

            # tiny warmup AllToAll: absorbs the CC stream's cold-start
            # latency (~40us on the first op) during proj0
            warm_in = dram.tile([8, 64], BF16, name="warm_in")
            warm_out = dram.tile([8, 64], BF16, name="warm_out")
            warm_sb = const.tile([8, 64], BF16, name="warm_sb")
            nc.vector.memset(warm_sb, 0.0)
            nc.sync.dma_start(out=warm_in, in_=warm_sb)
            nc.gpsimd.collective_compute(
                "AllToAll",
                mybir.AluOpType.bypass,
                replica_groups=[list(range(NCORES))],
                ins=[warm_in.opt()],
                outs=[warm_out.opt()],
            )


            # =================================================================
            # phase builders
            # =================================================================

            def proj_phase(b, post_tci=None, evict_on_act=True):
                """q/k/v projections + RMSNorm + RoPE + transposes for batch b.
                Writes QT/KT and V (bf16).  Transposes for token block i
                are interleaved into block i+1's matmul stream so the
                accumulating PSUM never stalls the PE; they land in the
                otherwise-idle third PSUM bank of block i's s3 tile.
                proj_tcis(b) returns the same work as 4 per-tci closures so
                the caller can interleave attention chunks between them."""
                for emit in proj_tcis(b, evict_on_act):
                    emit()
                    if post_tci is not None:
                        post_tci(emit.tci)

            def proj_tcis(b, evict_on_act=True):
                pending = []  # (accb, j, sb) awaiting eviction to QT/KT

                def emit_transpose(j, qr, accb, sb):
                    nc.tensor.transpose(
                        accb[:, j * 128 : (j + 1) * 128], qr[:, j], ident
                    )
                    pending.append((accb, j, sb))

                def emit_evict(accb, j, sb):
                    # plain proj phase: ACT has slack; interleaved with
                    # attention (exp hogs ACT's FIFO): use DVE instead
                    PT = KT[b] if j >= 2 else QT[b]
                    hl = j % 2
                    dst = PT[:, hl, sb * 128 : (sb + 1) * 128]
                    src = accb[:, j * 128 : (j + 1) * 128]
                    if evict_on_act:
                        nc.scalar.activation(out=dst, in_=src, func=AF.Copy)
                    else:
                        nc.vector.tensor_copy(out=dst, in_=src)

                tposes = []   # transposes awaiting emission

                def emit_tci(tci):
                    if (b, tci) not in xt_tiles:
                        load_xt(b, tci)
                    xt_sb = xt_tiles.pop((b, tci))
                    # prefetch next chunk
                    nxt = (b, tci + 1) if tci + 1 < S // TOKC else (b + 1, 0)
                    if nxt[0] < B and nxt not in xt_tiles:
                        load_xt(*nxt)
                    for tbl in range(TOKC // 128):
                        sb = tci * (TOKC // 128) + tbl   # seq block 0..15
                        s3 = p_s3.tile([128, 1024], F32, tag="s3")
                        pqA = s3[:, 0:384]
                        pqB = s3[:, 512:896]
                        for ci in range(16):
                            lhsT = xt_sb[:, ci, tbl * 128 : (tbl + 1) * 128]
                            nc.tensor.matmul(
                                pqA, lhsT=lhsT, rhs=w_ap(ci)[:, 0:384],
                                start=(ci == 0), stop=(ci == 15),
                            )
                            nc.tensor.matmul(
                                pqB, lhsT=lhsT, rhs=w_ap(ci)[:, 384:768],
                                start=(ci == 0), stop=(ci == 15),
                            )
                            # interleave previous block's transposes between
                            # accumulation steps (every 4th ci)
                            if ci % 4 == 3 and tposes:
                                emit_transpose(*tposes.pop(0))
                        while tposes:
                            emit_transpose(*tposes.pop(0))
                        while pending:
                            emit_evict(*pending.pop(0))

                        # free the psum banks quickly:
                        # qraw = [q_h0, q_h1, k_h0, k_h1] [128, 4, 128] fp32
                        qraw = norm_pool.tile([128, 4, 128], F32, tag="qraw")
                        if evict_on_act:
                            nc.scalar.activation(
                                out=qraw[:, 0:3], in_=pqA, func=AF.Copy
                            )
                            nc.scalar.activation(
                                out=qraw[:, 3], in_=s3[:, 512:640], func=AF.Copy
                            )
                            nc.scalar.activation(
                                out=V[b][:, sb, :], in_=s3[:, 640:896],
                                func=AF.Copy,
                            )
                        else:
                            nc.vector.tensor_copy(out=qraw[:, 0:3], in_=pqA)
                            nc.vector.tensor_copy(
                                out=qraw[:, 3], in_=s3[:, 512:640]
                            )
                            nc.vector.tensor_copy(
                                out=V[b][:, sb, :], in_=s3[:, 640:896]
                            )

                        # rms stats: rstd = rsqrt(mean(t^2)+eps), table-free
                        # Newton on DVE for (q_h0, q_h1, k_h0, k_h1)
                        sqs = norm_pool.tile([128, 4, 128], BF16, tag="sqs")
                        ssum = norm_pool.tile([128, 4], F32, tag="ssum")
                        nc.vector.tensor_tensor(
                            out=sqs, in0=qraw, in1=qraw, op=TT.mult
                        )
                        nc.vector.tensor_reduce(
                            out=ssum, in_=sqs, axis=mybir.AxisListType.X,
                            op=TT.add,
                        )
                        v_ = norm_pool.tile([128, 4], F32, tag="v_")
                        nc.vector.tensor_scalar(
                            out=v_, in0=ssum, scalar1=1.0 / DH, scalar2=EPS,
                            op0=TT.mult, op1=TT.add,
                        )
                        y = norm_pool.tile([128, 4], F32, tag="y")
                        t_ = norm_pool.tile([128, 4], F32, tag="t_")
                        u_ = norm_pool.tile([128, 4], F32, tag="u_")
                        # seed: y0 = bits(0x5f3759df - (bits(v) >> 1))
                        nc.vector.tensor_scalar(
                            out=y.bitcast(I32), in0=v_.bitcast(I32),
                            scalar1=1, scalar2=None,
                            op0=TT.logical_shift_right,
                        )
                        nc.vector.tensor_scalar(
                            out=y.bitcast(I32), in0=y.bitcast(I32),
                            scalar1=-1, scalar2=0x5F3759DF,
                            op0=TT.mult, op1=TT.add,
                        )
                        for it in range(2):  # Newton: y *= 1.5 - 0.5 v y^2
                            nc.vector.tensor_tensor(
                                out=t_, in0=y, in1=y, op=TT.mult
                            )
                            nc.vector.tensor_tensor(
                                out=t_, in0=t_, in1=v_, op=TT.mult
                            )
                            nc.vector.tensor_scalar(
                                out=u_, in0=t_, scalar1=-0.5, scalar2=1.5,
                                op0=TT.mult, op1=TT.add,
                            )
                            nc.vector.tensor_tensor(
                                out=y, in0=y, in1=u_, op=TT.mult
                            )

                        # apply norm in place (DVE; y broadcast over head_dim)
                        nc.vector.tensor_tensor(
                            out=qraw, in0=qraw,
                            in1=y[:, :, None].to_broadcast((128, 4, 128)),
                            op=TT.mult,
                        )
                        # rope: fp32 math, single bf16 rounding at the output
                        qr = norm_pool.tile([128, 4, 128], BF16, tag="qr")
                        qp = norm_pool.tile([128, 2, 2, 64], F32, tag="qp")
                        qn4 = qraw.rearrange("p (k h) d -> p k h d", k=2)
                        qr4 = qr.rearrange("p (k h) d -> p k h d", k=2)
                        lo = qn4[:, :, :, 0:64]
                        hi = qn4[:, :, :, 64:128]

                        def f(r):
                            return rope_sb[:, sb, :, None, r, :].to_broadcast(
                                (128, 2, 2, 64)
                            )

                        tmp = norm_pool.tile([128, 2, 2, 64], F32, tag="tmp")
                        nc.vector.tensor_tensor(
                            out=qp, in0=lo, in1=f(0), op=TT.mult
                        )
                        nc.vector.tensor_tensor(
                            out=tmp, in0=hi, in1=f(1), op=TT.mult
                        )
                        nc.vector.tensor_tensor(
                            out=qr4[:, :, :, 0:64], in0=qp, in1=tmp, op=TT.add
                        )
                        nc.vector.tensor_tensor(
                            out=qp, in0=lo, in1=f(2), op=TT.mult
                        )
                        nc.vector.tensor_tensor(
                            out=tmp, in0=hi, in1=f(3), op=TT.mult
                        )
                        nc.vector.tensor_tensor(
                            out=qr4[:, :, :, 64:128], in0=qp, in1=tmp, op=TT.add
                        )
                        # transpose quad for this block goes into an acc
                        # tile (bitcast to bf16: 4 x [128,128] in half a
                        # bank) so the s3 tile is released by the qraw/V
                        # evictions alone
                        acc = p_acc.tile([128, 512], F32, tag="acc")
                        accb = acc.bitcast(BF16)   # [128, 1024] bf16
                        for j in range(4):
                            tposes.append((j, qr, accb, sb))
                    # flush deferred transposes before handing the s3/acc
                    # rotation to whatever the caller interleaves next
                    while tposes:
                        emit_transpose(*tposes.pop(0))
                    while pending:
                        emit_evict(*pending.pop(0))

                emits = []
                for tci in range(S // TOKC):           # 4 chunks of 512 tokens
                    e = (lambda t: (lambda: emit_tci(t)))(tci)
                    e.tci = tci
                    emits.append(e)
                return emits

            def attn_chunk(ch):
                """attention for AG chunk ch (one batch, one q-range) +
                its AllGather."""
                b, q0, nt = CHUNKS[ch]
                for hl in range(HL):
                    po = p_acc.tile([128, 512], F32, tag="acc")
                    pden = p_acc.tile([128, 512], F32, tag="acc")
                    et = et_pool.tile([128, KB, 512], BF16, tag="et")
                    # den quad-sums: et summed over kb in groups of 4 on
                    # DVE (bf16 pair tree), so the softmax-denominator
                    # ones-matmul runs 4x fewer columns on the PE.
                    et5 = et.rearrange("p (k a b) q -> p k a b q", a=2, b=2)
                    den4 = [
                        den_pool.tile([128, 512], BF16, tag=f"den4_{k}",
                                      name=f"den4_{k}")
                        for k in range(4)
                    ]
                    # kb groups of 2; a small first group primes the
                    # PE<->ACT pipeline (PV can start after a 1-block exp)
                    groups = [(0, 1)] + [
                        (g, min(g + 2, KB)) for g in range(1, KB, 2)
                    ]

                    def scores(g0, g1):
                        s3 = p_s3.tile([128, 1024], F32, tag="s3")
                        for kb in range(g0, g1):
                            nc.tensor.matmul(
                                s3[:, (kb - g0) * 512 : (kb - g0) * 512 + nt],
                                lhsT=KT[b][:, hl, kb * 128 : (kb + 1) * 128],
                                rhs=QT[b][:, hl, q0 : q0 + nt],
                                start=True, stop=True,
                            )
                        return s3

                    def exp(s3, g0, g1):
                        # one ACT call over the whole group (cross-bank read)
                        nc.scalar.activation(
                            out=et[:, g0:g1, 0:nt],
                            in_=s3[:, 0 : (g1 - g0) * 512].rearrange(
                                "p (k n) -> p k n", k=g1 - g0
                            )[:, :, 0:nt],
                            func=AF.Exp,
                        )

                    def den_tree(k):
                        # quad k covers kb 4k..4k+3 (quad 3 only 12..14:
                        # kb15 streams straight into the pden matmul so
                        # the chunk tail never waits on the DVE)
                        ts1 = den_pool.tile([128, 2, 512], BF16, tag="ts1",
                                            name="ts1")
                        if k < 3:
                            nc.vector.tensor_tensor(
                                out=ts1[:, :, 0:nt],
                                in0=et5[:, k, :, 0, 0:nt],
                                in1=et5[:, k, :, 1, 0:nt],
                                op=TT.add,
                            )
                            nc.vector.tensor_tensor(
                                out=den4[k][:, 0:nt],
                                in0=ts1[:, 0, 0:nt],
                                in1=ts1[:, 1, 0:nt],
                                op=TT.add,
                            )
                        else:
                            nc.vector.tensor_tensor(
                                out=ts1[:, 0, 0:nt],
                                in0=et[:, 12, 0:nt],
                                in1=et[:, 13, 0:nt],
                                op=TT.add,
                            )
                            nc.vector.tensor_tensor(
                                out=den4[k][:, 0:nt],
                                in0=ts1[:, 0, 0:nt],
                                in1=et[:, 14, 0:nt],
                                op=TT.add,
                            )

                    def den_mm(k):
                        nc.tensor.matmul(
                            pden[:, 0:nt], lhsT=ones_mat,
                            rhs=den4[k][:, 0:nt],
                            start=(k == 0), stop=False,
                        )

                    def den_mm_last():
                        nc.tensor.matmul(
                            pden[:, 0:nt], lhsT=ones_mat,
                            rhs=et[:, 15, 0:nt],
                            start=False, stop=True,
                        )

                    def pv(g0, g1):
                        for kb in range(g0, g1):
                            nc.tensor.matmul(
                                po[:, 0:nt],
                                lhsT=V[b][:, kb, hl * 128 : (hl + 1) * 128],
                                rhs=et[:, kb, 0:nt],
                                start=(kb == 0), stop=(kb == KB - 1),
                            )

                    # quad k's tree can be emitted once exp covers its
                    # kbs; its matmul is deferred one group so the DVE
                    # result is ready when the PE reaches it.
                    tree_at = {5: 0, 9: 1, 13: 2, 15: 3}
                    prev = None
                    mm_q = []
                    for (g0, g1) in groups:
                        s3 = scores(g0, g1)
                        exp(s3, g0, g1)
                        if prev is not None:
                            pv(*prev)
                        if mm_q:
                            den_mm(mm_q.pop(0))
                        if g1 in tree_at:
                            den_tree(tree_at[g1])
                            mm_q.append(tree_at[g1])
                        prev = (g0, g1)
                    pv(*prev)
                    while mm_q:
                        den_mm(mm_q.pop(0))
                    den_mm_last()

                    # evict po/pden to SBUF fast (frees the acc banks),
                    # then normalize from SBUF off the PE-critical path
                    po_s = ot_pool.tile([128, 512], F32, tag="po_s")
                    den_s = ot_pool.tile([128, 512], F32, tag="den_s")
                    nc.vector.tensor_copy(out=den_s[:, 0:nt], in_=pden[:, 0:nt])
                    nc.vector.tensor_copy(out=po_s[:, 0:nt], in_=po[:, 0:nt])
                    if os.environ.get("ATTN_FAST_RECIP", "0") == "1":
                        recip = ot_pool.tile([128, 512], F32, tag="recip")
                        nc.vector.reciprocal_approx_fast(
                            out=recip[:, 0:nt], in_=den_s[:, 0:nt]
                        )
                    else:
                        recip = ot_pool.tile([128, 512], F32, tag="recip")
                        nc.vector.reciprocal(
                            out=recip[:, 0:nt], in_=den_s[:, 0:nt]
                        )
                    ot = ot_pool.tile([128, 512], BF16, tag="ot")
                    nc.vector.tensor_tensor(
                        out=ot[:, 0:nt], in0=po_s[:, 0:nt],
                        in1=recip[:, 0:nt], op=TT.mult,
                    )
                    # scatter into the A2A staging layout: dest block j
                    # gets this head's rows for core j's SH-token shard,
                    # at this chunk's SH-column offset within the batch
                    kk = ch % 4
                    nc.sync.dma_start(
                        out=ag_in[(b, hl)].rearrange(
                            "(j f) (k u) -> f j k u", j=NCORES, k=4
                        )[:, :, kk],
                        in_=ot[:, 0:nt].rearrange("p (j u) -> p j u", j=NCORES),
                    )

            otf_b = {}

            def wo_load(b, hl):
                """land a (batch, local-head) AllToAll result (8 cores x
                128 features for my TPC tokens).  sync queue: a wo_load
                waits on its AllToAll, and the gpsimd queue (collective
                triggers) must not stall on it."""
                t = wo_in.tile([128, NCORES, TPC], BF16, tag=f"otf{hl}",
                               name=f"otf_b{b}_{hl}")
                ag_r = ag_out[(b, hl)].rearrange("(c p) u -> p c u", p=128)
                nc.sync.dma_start(out=t, in_=ag_r)
                otf_b[(b, hl)] = t

            wot_tiles = {}

            def wot_load(key, e):
                """stream an of-eighth (256 output features) of wo^T into
                an xt-pool tile (free after proj1).  gpsimd queue: idle
                during the wo drain, keeping the sync queue for ot/out."""
                t = xt_pool.tile([128, 16, TOKC], BF16, tag="xt")
                nc.scalar.dma_start(
                    out=t, in_=wot[:, :, e * 256 : (e + 1) * 256]
                )
                wot_tiles[key] = t

            def wo_mm8(b, e):
                """output projection for one (batch, of-eighth): all of my
                TPC tokens of batch b against 256 output features.  The
                contraction runs hl=0 feature blocks first so it can start
                as soon as the first A2A half lands."""
                w = wot_tiles.pop((b, e))
                pw = p_acc.tile([128, 512], F32, tag="acc")
                for ofb in range(2):
                    for m, ci in enumerate(
                        [2 * c + hl for hl in range(HL) for c in range(8)]
                    ):
                        nc.tensor.matmul(
                            pw[:, ofb * TPC : (ofb + 1) * TPC],
                            lhsT=w[:, ci, ofb * 128 : (ofb + 1) * 128],
                            rhs=otf_b[(b, ci % 2)][:, ci // 2, :],
                            start=(m == 0), stop=(m == 15),
                        )
                osb = ot_pool.tile([128, 512], F32, tag="osb")
                nc.vector.tensor_copy(out=osb, in_=pw)
                nc.sync.dma_start(
                    out=out_ext[
                        e * 256 : (e + 1) * 256, b * TPC : (b + 1) * TPC
                    ].rearrange("(a p) t -> p a t", a=2),
                    in_=osb.rearrange("p (a t) -> p a t", a=2),
                )

            # ---- emission order (controls per-engine instruction order) ----
            # proj0 -> attn0 -> proj1 -> attn1: batch 0's AllGathers run
            # during attn0/proj1, batch 1's during attn1, so the comm stream
            # spans most of the kernel and only the last small chunk's AG is
            # exposed.  wo for chunk c is scheduled well after its AllGather
            # lands (otf prefetched one hook earlier; never before the AG's
            # trigger is emitted, else the gpsimd queue deadlocks).
            # proj0, then batch-0 attention chunks interleaved with proj1's
            # token chunks: the AllGather stream starts ~40us after proj0
            # and stays busy through proj1, and batch-1 attention (whose 6
            # AGs are the serial tail) starts ~80us earlier than running
            # attn0 and proj1 back-to-back would allow.  wo for chunk c is
            # scheduled 2+ chunks after its AllGather is triggered.
            proj_phase(0)
            p1 = proj_tcis(1)
            attn_chunk(0); p1[0](); p1[1]()
            attn_chunk(1); p1[2](); p1[3]()
            attn_chunk(2); a2a_warm(); p1[4](); p1[5]()
            attn_chunk(3); a2a(0, 0); a2a(0, 1); p1[6](); p1[7]()
            # batch-1 attention back-to-back; batch-0's A2As (~30us CC
            # ops) land under it.  All wo work drains after the last
            # chunk so batch-1's A2As trigger as early as possible; the
            # hl=1 half hides under wo's hl=0 contraction steps.
            attn_chunk(4)
            wo_load(0, 0); wo_load(0, 1)
            wot_load((0, 0), 0); wot_load((0, 1), 1)
            attn_chunk(5)
            attn_chunk(6); a2a_warm()
            attn_chunk(7); a2a(1, 0); a2a(1, 1)
            for e in range(8):
                wo_mm8(0, e)
                if e < 6:
                    wot_load((0, e + 2), e + 2)
                else:
                    wot_load((1, e - 6), e - 6)
                if e == 5:
                    # sync reaches here after b0's e<=5 out stores; the
                    # A2A(1,*) waits no longer gate anything behind them
                    wo_load(1, 0); wo_load(1, 1)
            for e in range(8):
                wo_mm8(1, e)
                if e < 6:
                    wot_load((1, e + 2), e + 2)

    return nc


def _prep_inputs(x, rope_emb, wq, wk, wv, wo, q_norm_w, k_norm_w):
    """Host-side shard prep: per-core input maps."""
    bf = ml_dtypes.bfloat16
    X = np.ascontiguousarray(x.reshape(TOK, DM))
    xt = np.ascontiguousarray(X.T).astype(bf)  # [DM, TOK]

    gamma = 1.0 / np.sqrt(DH)
    qw = np.asarray(q_norm_w, np.float32)
    kw = np.asarray(k_norm_w, np.float32)
    fr = np.asarray(rope_emb, np.float32)[:, 0]  # [S, 64, 2, 2]

    def rope_pack(w, scale):
        # F[r] for r=(i,l): out[i*64+j] += F[i,l][s,j] * t[l*64+j], t = norm*w
        F = np.empty((S, 4, 64), np.float32)
        F[:, 0] = fr[:, :, 0, 0] * w[None, :64] * scale
        F[:, 1] = fr[:, :, 0, 1] * w[None, 64:] * scale
        F[:, 2] = fr[:, :, 1, 0] * w[None, :64] * scale
        F[:, 3] = fr[:, :, 1, 1] * w[None, 64:] * scale
        return F

    rope_all = np.stack([rope_pack(qw, gamma), rope_pack(kw, 1.0)], axis=1)
    # [S, 2, 4, 64] -> partition-major [128, SB, 2, 4, 64]
    rope_pm = np.ascontiguousarray(
        rope_all.reshape(SB, 128, 2, 4, 64).transpose(1, 0, 2, 3, 4)
    ).astype(bf)

    def pmajor(a):
        # [DM, F] -> [128, 16, F] with dm = c*128 + p
        return np.ascontiguousarray(
            a.reshape(16, 128, a.shape[1]).transpose(1, 0, 2)
        )

    wot_full = pmajor(np.ascontiguousarray(wo.T).astype(bf))  # [128,16,2048]
    in_maps = []
    for c in range(NCORES):
        rows = slice(c * FC, (c + 1) * FC)
        wqkv = np.concatenate(
            [wq[rows].T, wk[rows].T, wv[rows].T], axis=1
        ).astype(bf)  # [DM, 768]
        in_maps.append(
            {
                "xt": xt,
                "wqkv": pmajor(wqkv),
                "wot": wot_full,
                "rope": rope_pm,
            }
        )
    return in_maps


_CACHE = {}


def kernel(x, rope_emb, wq, wk, wv, wo, q_norm_w, k_norm_w):
    global LAST_EXEC_NS, LAST_RES
    x = np.asarray(x, np.float32)
    rope_emb = np.asarray(rope_emb, np.float32)
    wq = np.asarray(wq, np.float32)
    wk = np.asarray(wk, np.float32)
    wv = np.asarray(wv, np.float32)
    wo = np.asarray(wo, np.float32)
    q_norm_w = np.asarray(q_norm_w, np.float32)
    k_norm_w = np.asarray(k_norm_w, np.float32)
    _patch_tile_drain()
    _patch_ldw_opt()
    _register_ntff_hook()
    from concourse.bass_utils import run_bass_kernel_spmd

    if "nc" not in _CACHE:
        nc = _build()
        if os.environ.get("ATTN_STRIP_INCS", "0") == "1":
            _strip_redundant_incs(nc)
        _legalize_waits(nc)
        _CACHE["nc"] = nc
    nc = _CACHE["nc"]

    in_maps = _prep_inputs(x, rope_emb, wq, wk, wv, wo, q_norm_w, k_norm_w)
    trace = os.environ.get("ATTN_TRACE", "0") == "1"
    res = run_bass_kernel_spmd(
        nc, in_maps, core_ids=list(range(NCORES)), trace=trace
    )
    LAST_EXEC_NS = res.exec_time_ns
    LAST_RES = res

    # out_ext is [DM, B*TPC] per core: all 2048 output features for the
    # core's token shards (SH consecutive tokens per 512-token chunk).
    out = np.empty((B, S, DM), np.float32)
    for c in range(NCORES):
        oc = res.results[c]["out"]  # [DM, B*TPC]
        for b in range(B):
            for k in range(4):       # chunk k within batch
                sl = oc[:, b * TPC + k * SH : b * TPC + (k + 1) * SH]
                q0 = k * 512 + c * SH
                out[b, q0 : q0 + SH, :] = sl.T
    return np.ascontiguousarray(out)

